# revision 20
# baseline (speedup 1.0000x reference)
"""4-layer tanh RNN on 8 Trainium2 NeuronCores.

Strategy: 4-stage layer pipeline x 2-way batch split (core c = layer c//2,
batch half c%2). Time is processed in T=32-step blocks over ROUNDS =
NB + 2*(NL-1) rounds (lag-2 consumption: a block produced in round r
travels through an AllGather during round r+1 and is consumed in round
r+2, so the collective is fully off the critical path).

Per round, a core runs the 32-step recurrence for its layer reading the
step input projection xw directly from PSUM (pre-accumulated there by the
previous round's interleaved projection matmuls — no vector add on the
step critical path: matmuls accumulate onto xw in PSUM, one tanh per
(step, m-half) reads PSUM and writes the bf16 h tile). The projection of
the NEXT round's block (gathered early via the lag-2 AG) is interleaved
into the second half of the round's steps to fill PE gaps. Bias is folded
in as one extra rank-1 matmul per (m, parity) using a ones-vector rhs.

The AllGather carries one P-row block per core (plus a junk row for
scatter targets): cores 0-5 contribute their output block for the next
stage; cores 6,7 contribute the stage-0 input feed (host-staged X blocks),
which cores 0,1 gather. Per-core *data* (gather/scatter indices,
carry/init masks) encodes each core's role; the instruction stream is
SPMD-uniform.

Compute dtype bf16 (fp8 weights tested: 12% end-to-end error, rejected),
fp32 PSUM accumulation, fp32 tanh, bf16 outputs converted on host.
"""
import sys
import numpy as np

if "/opt/trn_rl_repo" not in sys.path:
    sys.path.insert(0, "/opt/trn_rl_repo")

import ml_dtypes

BF = ml_dtypes.bfloat16

B, L, D, NL = 16, 512, 1024, 4
P = 128
KT = D // P          # 8 contraction tiles
MT = D // P          # 8 output tiles
BC = B // 2          # 8 batch rows per core
T = 32               # timesteps per block
NB = L // T          # 16 blocks
LAG = 2              # rounds from production to consumption
ROUNDS = NB + LAG * (NL - 1)   # 22
N_CORES = 8
BLK_COLS = KT * T * BC         # 2048; col = t*(KT*BC) + k*BC + bc
RP = P + 1                     # AG rows per core (row P = scatter junk)
DUM_PER_SLOT = 21              # target filler MMs per step slot (real+dummy)

_cache = {}


def _build():
    import concourse.bass as bass
    import concourse.mybir as mybir
    import concourse.tile as tile
    from concourse import bacc
    from concourse.tile import add_dep_helper

    F32 = mybir.dt.float32
    BF16 = mybir.dt.bfloat16
    I32 = mybir.dt.int32
    Tanh = mybir.ActivationFunctionType.Tanh

    nc = bacc.Bacc("TRN2", target_bir_lowering=False, debug=False,
                   num_devices=N_CORES)

    # ---- I/O ----
    whT = nc.dram_tensor("whT", [P, KT * MT * P], BF16, kind="ExternalInput")
    wxT = nc.dram_tensor("wxT", [P, KT * MT * P], BF16, kind="ExternalInput")
    biasT = nc.dram_tensor("biasT", [P, MT * P], BF16, kind="ExternalInput")
    carry = nc.dram_tensor("carry", [ROUNDS, P, KT * BC], mybir.dt.uint8,
                           kind="ExternalInput")
    cinit = nc.dram_tensor("cinit", [ROUNDS, P, KT * BC], BF16,
                           kind="ExternalInput")
    gidx = nc.dram_tensor("gidx", [P, 1], I32, kind="ExternalInput")
    sidx_blk = nc.dram_tensor("sidx_blk", [P, 1], I32, kind="ExternalInput")
    sidx_feed = nc.dram_tensor("sidx_feed", [P, 1], I32, kind="ExternalInput")
    x0t = nc.dram_tensor("x0t", [ROUNDS, P, BLK_COLS], BF16,
                         kind="ExternalInput")
    ag_init = nc.dram_tensor("ag_init", [LAG, P, BLK_COLS], BF16,
                             kind="ExternalInput")
    out = nc.dram_tensor("out", [ROUNDS, P, BLK_COLS], BF16,
                         kind="ExternalOutput")

    NAG = ROUNDS - LAG  # 20 rounds with collectives (x2 half-block AGs)
    HC = BLK_COLS // 2  # 1024 cols per half-block (t<16 / t>=16)
    ag_ins = [[nc.dram_tensor(f"ag_in_{half}_{r}", [RP, HC], BF16)
               for r in range(NAG)] for half in range(2)]
    ag_outs = [[nc.dram_tensor(f"ag_out_{half}_{r}", [N_CORES * RP, HC], BF16,
                               addr_space="Shared")
                for r in range(NAG)] for half in range(2)]

    with tile.TileContext(nc) as tc:
        with (
            tc.tile_pool(name="const", bufs=1) as cpool,
            tc.tile_pool(name="hs", bufs=2) as hspool,
            tc.tile_pool(name="ps", bufs=1, space="PSUM") as pspool,
        ):
            wh_sb = cpool.tile([P, KT, MT, P], BF16, tag="wh")
            nc.sync.dma_start(wh_sb[:], whT.ap().rearrange(
                "p (k m q) -> p k m q", k=KT, m=MT))
            wx_sb = cpool.tile([P, KT, MT, P], BF16, tag="wx")
            nc.sync.dma_start(wx_sb[:], wxT.ap().rearrange(
                "p (k m q) -> p k m q", k=KT, m=MT))
            bias_sb = cpool.tile([P, MT, P], BF16, tag="bias")
            nc.sync.dma_start(bias_sb[:], biasT.ap().rearrange(
                "p (m q) -> p m q", m=MT))
            carry_sb = cpool.tile([P, ROUNDS, KT * BC], mybir.dt.uint8,
                                  tag="carry")
            nc.sync.dma_start(carry_sb[:], carry.ap().rearrange("r p c -> p r c"))
            cinit_sb = cpool.tile([P, ROUNDS, KT * BC], BF16, tag="cinit")
            nc.sync.dma_start(cinit_sb[:], cinit.ap().rearrange("r p c -> p r c"))
            gidx_sb = cpool.tile([P, 1], I32, tag="gidx")
            nc.sync.dma_start(gidx_sb[:], gidx[:])
            sblk_sb = cpool.tile([P, 1], I32, tag="sblk")
            nc.sync.dma_start(sblk_sb[:], sidx_blk[:])
            sfeed_sb = cpool.tile([P, 1], I32, tag="sfeed")
            nc.sync.dma_start(sfeed_sb[:], sidx_feed[:])
            ones_sb = cpool.tile([P, P], BF16, tag="ones")
            nc.vector.memset(ones_sb[:], 1.0)

            # h output tiles, double-buffered by round parity.
            # layout [P, t, m, bc]
            curA = cpool.tile([P, T, MT, BC], BF16, tag="curA")
            curB = cpool.tile([P, T, MT, BC], BF16, tag="curB")
            nc.vector.memset(curA[:], 0.0)
            nc.vector.memset(curB[:], 0.0)

            # xw PSUM tiles: [P, bank(4), t'(8), m(8), bc(8)] fp32.
            # bank = t%2 + 2*(t>=16); t' = (t%16)//2. Consecutive steps hit
            # different banks so ACT(t) reads never collide with MM(t+1)
            # writes; 4 banks per tile, double-buffered = all 8 banks.
            psA = pspool.tile([P, 4, T // 4, MT, BC], F32, tag="psA")
            psB = pspool.tile([P, 4, T // 4, MT, BC], F32, tag="psB")

            xblks = [cpool.tile([P, T, KT, BC], BF16, tag=f"xblk{i}", name=f"xblk{i}")
                     for i in range(2)]
            feeds = [cpool.tile([P, BLK_COLS], BF16, tag=f"feed{i}", name=f"feed{i}")
                     for i in range(2)]

            # ---- projection emission helper -------------------------------
            def proj_jobs(r):
                """Thunks emitting proj/bias MMs for round r's xw.

                Writes ps[r%2]; reads xblk tile of round r. Ordered so the
                always-ready bias MMs come first (they fill PE slots while
                the gathers land), then lo-half proj, then hi-half proj.
                The first bias MM per bank has start=True (clears the bank);
                everything else accumulates and depends on that clear.
                """
                ps = psA if r % 2 == 0 else psB
                xb = xblks[r % 2]
                clear_mm = [None, None, None, None]

                def mk_bias(b, m):
                    # rank-1: lhsT = bias tile (only partition 0 nonzero),
                    # rhs = ones => writes b[m*128+o] to every (t', bc) col
                    def emit():
                        mm = nc.tensor.matmul(
                            ps[:, b, :, m, :],
                            bias_sb[:, m, :],
                            ones_sb[:, :T // 4 * BC],
                            start=(clear_mm[b] is None),
                            stop=False,
                            skip_group_check=True,
                        )
                        if clear_mm[b] is None:
                            clear_mm[b] = mm
                        else:
                            add_dep_helper(mm.ins, clear_mm[b].ins, sync=False,
                                           reason="after bank clear")
                        return mm
                    return emit

                def mk(H, par, m, k):
                    def emit():
                        b = 2 * H + par
                        mm = nc.tensor.matmul(
                            ps[:, b, :, m, :],
                            wx_sb[:, k, m, :],
                            xb[:, 16 * H + par:16 * (H + 1):2, k, :],
                            start=False,
                            stop=False,
                            skip_group_check=True,
                        )
                        add_dep_helper(mm.ins, clear_mm[b].ins, sync=False,
                                       reason="after bank clear")
                        return mm
                    return emit

                jobs = []
                for b in range(4):
                    for m in range(MT):
                        jobs.append(mk_bias(b, m))
                for H in range(2):
                    for par in range(2):
                        for m in range(MT):
                            for k in range(KT):
                                jobs.append(mk(H, par, m, k))
                return jobs

            # ---- prologue: gather + project block for round 0 -------------
            nc.sync.dma_start(xblks[0][:], ag_init[0].rearrange(
                "p (t k c) -> p t k c", t=T, k=KT))
            for j in proj_jobs(0):
                j()

            def contribute(r, half):
                """Scatter this round's half-block + trigger its AG."""
                cur = curA if r % 2 == 0 else curB
                lo, hi = half * HC, (half + 1) * HC
                tlo, thi = half * (T // 2), (half + 1) * (T // 2)
                sc_blk = nc.gpsimd.indirect_dma_start(
                    out=ag_ins[half][r][:],
                    out_offset=bass.IndirectOffsetOnAxis(
                        ap=sblk_sb[:, :1], axis=0),
                    in_=cur[:, tlo:thi, :, :].rearrange("p t m c -> p (t m c)"),
                    in_offset=None,
                )
                cc = nc.gpsimd.collective_compute(
                    "AllGather",
                    mybir.AluOpType.bypass,
                    replica_groups=[list(range(N_CORES))],
                    ins=[ag_ins[half][r][:]],
                    outs=[ag_outs[half][r][:]],
                )
                add_dep_helper(cc.ins, sc_blk.ins, sync=True,
                               reason="AG after block scatter")
                add_dep_helper(cc.ins, sc_feeds[half].ins, sync=True,
                               reason="AG after feed scatter")
                cc_lists[half].append(cc)

            cc_lists = [[], []]
            sc_feeds = [None, None]
            for r in range(ROUNDS):
                cur = curA if r % 2 == 0 else curB
                prev = curB if r % 2 == 0 else curA
                ps = psA if r % 2 == 0 else psB

                # ---- early, off critical path ----
                # feed halves for this round's AG contributions
                if r < NAG:
                    nc.sync.dma_start(feeds[r % 2][:], x0t[r])
                    for half in range(2):
                        sc_feeds[half] = nc.gpsimd.indirect_dma_start(
                            out=ag_ins[half][r][:],
                            out_offset=bass.IndirectOffsetOnAxis(
                                ap=sfeed_sb[:, :1], axis=0),
                            in_=feeds[r % 2][:, half * HC:(half + 1) * HC],
                            in_offset=None,
                        )
                # gather next round's input block (two halves)
                if r + 1 < ROUNDS:
                    nxb = xblks[(r + 1) % 2]
                    if r + 1 < LAG:
                        nc.sync.dma_start(nxb[:], ag_init[r + 1].rearrange(
                            "p (t k c) -> p t k c", t=T, k=KT))
                    else:
                        for half in range(2):
                            tlo, thi = half * (T // 2), (half + 1) * (T // 2)
                            g = nc.gpsimd.indirect_dma_start(
                                out=nxb[:, tlo:thi, :, :].rearrange(
                                    "p t k c -> p (t k c)"),
                                out_offset=None,
                                in_=ag_outs[half][r - 1][:],
                                in_offset=bass.IndirectOffsetOnAxis(
                                    ap=gidx_sb[:, :1], axis=0),
                            )
                            add_dep_helper(g.ins, cc_lists[half][r - 1].ins,
                                           sync=True, reason="gather after AG")

                # ---- h_start = carry ? prev block tail : cinit ----
                hstart = hspool.tile([P, KT * BC], BF16, tag="hs")
                nc.vector.tensor_copy(hstart[:], cinit_sb[:, r])
                nc.vector.copy_predicated(
                    hstart[:], carry_sb[:, r],
                    prev[:, T - 1, :, :].rearrange("p m c -> p (m c)"))

                # ---- 32 recurrence steps; proj(r+1) MMs fill every slot ----
                # The scheduler floats independent MMs, so fillers are pinned
                # into their slot with sync=False ordering edges: fillers
                # follow the slot's first step-MM; the next step's first MM
                # follows the slot's last filler. Fillers then execute in the
                # PE FIFO while ACT(t) completes, hiding its latency.
                # Slot ranges keep each filler's gather strictly older than
                # its slot so it can never head-of-line-block the FIFO.
                jobs = proj_jobs(r + 1) if r + 1 < ROUNDS else []
                ji = 0
                anchor = None
                for t in range(T):
                    bnk = t % 2 + 2 * (t >= T // 2)
                    reg = ps[:, bnk, (t % (T // 2)) // 2, :, :]
                    first_mm = None
                    for k in range(KT):
                        if t == 0:
                            rhs = hstart[:, k * BC:(k + 1) * BC]
                        else:
                            rhs = cur[:, t - 1, k, :]
                        for m in range(MT):
                            mm = nc.tensor.matmul(
                                reg[:, m, :],
                                wh_sb[:, k, m, :],
                                rhs,
                                start=False,
                                stop=False,
                                skip_group_check=True,
                            )
                            if first_mm is None:
                                first_mm = mm
                                if anchor is not None:
                                    add_dep_helper(mm.ins, anchor.ins,
                                                   sync=False,
                                                   reason="after fillers")
                    nc.scalar.activation(cur[:, t, :, :], reg, Tanh)
                    # fillers for this slot: bias in steps 1-4, lo-half proj
                    # in 4-17, hi-half proj from 18 (gather timing safety)
                    nfill = 0
                    if t >= 1:
                        nj = min(9 if t >= 4 else 8, len(jobs) - ji)
                        if t < 17:
                            nj = min(nj, 32 + 128 - ji)  # bias+lo only
                        for _ in range(max(nj, 0)):
                            f = jobs[ji]()
                            add_dep_helper(f.ins, first_mm.ins, sync=False,
                                           reason="filler in slot")
                            anchor = f
                            ji += 1
                            nfill += 1
                    # dummy fillers top the slot up so the PE never idles
                    # while ACT(t) completes (also keeps the HAM clock warm).
                    # They accumulate garbage into the previous step's
                    # already-consumed xw region: ACT(t-1) is provably done
                    # (this step's MMs waited on it), ACT(t) reads a
                    # different bank, so no PSUM collision.
                    if t >= 1:
                        bprev = (t - 1) % 2 + 2 * ((t - 1) >= T // 2)
                        dreg = ps[:, bprev, ((t - 1) % (T // 2)) // 2, :, :]
                        for di in range(max(DUM_PER_SLOT - nfill, 0)):
                            f = nc.tensor.matmul(
                                dreg[:, di % MT, :],
                                wh_sb[:, 0, di % MT, :],
                                ones_sb[:, :BC],
                                start=False,
                                stop=False,
                                skip_group_check=True,
                            )
                            add_dep_helper(f.ins, first_mm.ins, sync=False,
                                           reason="dummy filler")
                            anchor = f
                    if t == T // 2 - 1 and r < NAG:
                        contribute(r, 0)   # lo half done -> AG it mid-round
                while ji < len(jobs):
                    f = jobs[ji]()
                    add_dep_helper(f.ins, first_mm.ins, sync=False,
                                   reason="filler in slot")
                    ji += 1

                # ---- write output block (bf16; host converts) ----
                nc.sync.dma_start(
                    out[r], cur[:].rearrange("p t m c -> p (t m c)"))

                if r < NAG:
                    contribute(r, 1)
    nc.compile()
    return nc


def _prep_inputs(X, h0s, W, b):
    """Build the 8 per-core input maps."""
    in_maps = []

    def xb_layout(Xj):
        # [bc, L, d] -> per block [P, (t, k, bc)]
        A = Xj.reshape(BC, NB, T, KT, P)          # [bc, nb, t, k, p]
        A = A.transpose(1, 4, 2, 3, 0)            # [nb, p, t, k, bc]
        return np.ascontiguousarray(A.reshape(NB, P, BLK_COLS)).astype(BF)

    for c in range(N_CORES):
        s, j = c // 2, c % 2
        Wl = np.asarray(W[s], dtype=np.float32)
        Wx, Wh = Wl[:, :D], Wl[:, D:]

        def tiles(M):  # M: [e, d] -> lhsT tiles [p, (k, m, q)]
            A = M.reshape(MT, P, KT, P)           # [m, q, k, p]
            return np.ascontiguousarray(
                A.transpose(3, 2, 0, 1).reshape(P, KT * MT * P)).astype(BF)

        whT = tiles(Wh)
        wxT = tiles(Wx)
        biasT = np.zeros((P, MT, P), np.float32)
        biasT[0] = np.asarray(b[s], np.float32).reshape(MT, P)
        biasT = biasT.reshape(P, MT * P).astype(BF)

        hin = np.asarray(h0s[s, BC * j:BC * (j + 1)], np.float32)  # [bc, d]
        hinit = np.ascontiguousarray(
            hin.reshape(BC, KT, P).transpose(2, 1, 0).reshape(P, KT * BC)
        ).astype(BF)

        carry = np.zeros((ROUNDS, P, KT * BC), np.uint8)
        cinit = np.zeros((ROUNDS, P, KT * BC), BF)
        for r in range(ROUNDS):
            if r > LAG * s:
                carry[r] = 1
            elif r == LAG * s:
                cinit[r] = hinit

        x0t = np.zeros((ROUNDS, P, BLK_COLS), BF)
        ag_init = np.zeros((LAG, P, BLK_COLS), BF)
        if s == 0:
            Xb = xb_layout(np.asarray(X[BC * j:BC * (j + 1)], np.float32))
            ag_init[0] = Xb[0]
            ag_init[1] = Xb[1]
        if s == 3:
            # cores 6,7 carry the stage-0 feed for half j: block r+2 at round r
            Xb = xb_layout(np.asarray(X[BC * j:BC * (j + 1)], np.float32))
            for r in range(ROUNDS):
                if r + 2 < NB:
                    x0t[r] = Xb[r + 2]

        if s == 0:
            gidx = ((6 + c) * RP + np.arange(P, dtype=np.int32)).reshape(P, 1)
        else:
            gidx = ((c - 2) * RP + np.arange(P, dtype=np.int32)).reshape(P, 1)
        if s == 3:
            sidx_blk = np.full((P, 1), P, np.int32)      # junk row
            sidx_feed = np.arange(P, dtype=np.int32).reshape(P, 1)
        else:
            sidx_blk = np.arange(P, dtype=np.int32).reshape(P, 1)
            sidx_feed = np.full((P, 1), P, np.int32)     # junk row

        in_maps.append({
            "whT": whT, "wxT": wxT, "biasT": biasT,
            "carry": carry, "cinit": cinit,
            "gidx": gidx, "sidx_blk": sidx_blk, "sidx_feed": sidx_feed,
            "x0t": x0t, "ag_init": ag_init,
        })
    return in_maps


def _extract(results):
    """Assemble full output [B, L, D] from stage-3 cores (6, 7)."""
    Y = np.empty((B, L, D), np.float32)
    r0 = LAG * (NL - 1)
    for j in range(2):
        o = results[6 + j]["out"][r0:r0 + NB]            # [nb, p, (t m c)]
        o = o.reshape(NB, P, T, MT, BC).astype(np.float32)
        o = o.transpose(4, 0, 2, 3, 1)                   # [bc, nb, t, m, p]
        Y[BC * j:BC * (j + 1)] = o.reshape(BC, L, D)
    return Y


def kernel(X, h0s, W, b, _trace=False):
    from concourse.bass_utils import run_bass_kernel_spmd

    if "nc" not in _cache:
        _cache["nc"] = _build()
    nc = _cache["nc"]
    in_maps = _prep_inputs(np.asarray(X), np.asarray(h0s), np.asarray(W),
                           np.asarray(b))
    res = run_bass_kernel_spmd(nc, in_maps, core_ids=list(range(N_CORES)),
                               trace=_trace)
    _cache["last_results"] = res
    return _extract(res.results)


# revision 23
# speedup vs baseline: 1.3684x; 1.3684x over previous
"""4-layer tanh RNN on 8 Trainium2 NeuronCores.

Strategy: 4-stage layer pipeline x 2-way batch split (core c = layer c//2,
batch half c%2). Time is processed in T=32-step blocks over ROUNDS =
NB + 2*(NL-1) rounds (lag-2 consumption: a block produced in round r
travels through an AllGather during round r+1 and is consumed in round
r+2, so the collective is fully off the critical path).

Per round, a core runs the 32-step recurrence for its layer reading the
step input projection xw directly from PSUM (pre-accumulated there by the
previous round's interleaved projection matmuls — no vector add on the
step critical path: matmuls accumulate onto xw in PSUM, one tanh per
(step, m-half) reads PSUM and writes the bf16 h tile). The projection of
the NEXT round's block (gathered early via the lag-2 AG) is interleaved
into the second half of the round's steps to fill PE gaps. Bias is folded
in as one extra rank-1 matmul per (m, parity) using a ones-vector rhs.

The AllGather carries one P-row block per core (plus a junk row for
scatter targets): cores 0-5 contribute their output block for the next
stage; cores 6,7 contribute the stage-0 input feed (host-staged X blocks),
which cores 0,1 gather. Per-core *data* (gather/scatter indices,
carry/init masks) encodes each core's role; the instruction stream is
SPMD-uniform.

Compute dtype bf16 (fp8 weights tested: 12% end-to-end error, rejected),
fp32 PSUM accumulation, fp32 tanh, bf16 outputs converted on host.
"""
import sys
import numpy as np

if "/opt/trn_rl_repo" not in sys.path:
    sys.path.insert(0, "/opt/trn_rl_repo")

import ml_dtypes

BF = ml_dtypes.bfloat16

B, L, D, NL = 16, 512, 1024, 4
P = 128
KT = D // P          # 8 contraction tiles
MT = D // P          # 8 output tiles
BC = B // 2          # 8 batch rows per core
T = 16               # timesteps per block
NB = L // T          # 16 blocks
LAG = 2              # rounds from production to consumption
ROUNDS = NB + LAG * (NL - 1)   # 22
N_CORES = 8
BLK_COLS = KT * T * BC         # 2048; col = t*(KT*BC) + k*BC + bc
RP = P + 1                     # AG rows per core (row P = scatter junk)
DUM_PER_SLOT = 0               # dummy filler top-up per slot (0 = disabled;
                               # measured: dummies re-inflate the ACT waits)

_cache = {}


def _build():
    import concourse.bass as bass
    import concourse.mybir as mybir
    import concourse.tile as tile
    from concourse import bacc
    from concourse.tile import add_dep_helper

    F32 = mybir.dt.float32
    BF16 = mybir.dt.bfloat16
    I32 = mybir.dt.int32
    Tanh = mybir.ActivationFunctionType.Tanh

    nc = bacc.Bacc("TRN2", target_bir_lowering=False, debug=False,
                   num_devices=N_CORES)

    # ---- I/O ----
    whT = nc.dram_tensor("whT", [P, KT * MT * P], BF16, kind="ExternalInput")
    wxT = nc.dram_tensor("wxT", [P, KT * MT * P], BF16, kind="ExternalInput")
    biasT = nc.dram_tensor("biasT", [P, MT * P], BF16, kind="ExternalInput")
    carry = nc.dram_tensor("carry", [ROUNDS, P, KT * BC], mybir.dt.uint8,
                           kind="ExternalInput")
    cinit = nc.dram_tensor("cinit", [ROUNDS, P, KT * BC], BF16,
                           kind="ExternalInput")
    gidx = nc.dram_tensor("gidx", [P, 1], I32, kind="ExternalInput")
    sidx_blk = nc.dram_tensor("sidx_blk", [P, 1], I32, kind="ExternalInput")
    sidx_feed = nc.dram_tensor("sidx_feed", [P, 1], I32, kind="ExternalInput")
    x0t = nc.dram_tensor("x0t", [ROUNDS, P, BLK_COLS], BF16,
                         kind="ExternalInput")
    ag_init = nc.dram_tensor("ag_init", [LAG, P, BLK_COLS], BF16,
                             kind="ExternalInput")
    out = nc.dram_tensor("out", [ROUNDS, P, BLK_COLS], BF16,
                         kind="ExternalOutput")

    NAG = ROUNDS - LAG  # 20 rounds with collectives (x2 half-block AGs)
    HC = BLK_COLS // 2  # 1024 cols per half-block (t<16 / t>=16)
    ag_ins = [[nc.dram_tensor(f"ag_in_{half}_{r}", [RP, HC], BF16)
               for r in range(NAG)] for half in range(2)]
    ag_outs = [[nc.dram_tensor(f"ag_out_{half}_{r}", [N_CORES * RP, HC], BF16,
                               addr_space="Shared")
                for r in range(NAG)] for half in range(2)]

    with tile.TileContext(nc) as tc:
        with (
            tc.tile_pool(name="const", bufs=1) as cpool,
            tc.tile_pool(name="hs", bufs=2) as hspool,
            tc.tile_pool(name="ps", bufs=1, space="PSUM") as pspool,
        ):
            wh_sb = cpool.tile([P, KT, MT, P], BF16, tag="wh")
            nc.sync.dma_start(wh_sb[:], whT.ap().rearrange(
                "p (k m q) -> p k m q", k=KT, m=MT))
            wx_sb = cpool.tile([P, KT, MT, P], BF16, tag="wx")
            nc.sync.dma_start(wx_sb[:], wxT.ap().rearrange(
                "p (k m q) -> p k m q", k=KT, m=MT))
            bias_sb = cpool.tile([P, MT, P], BF16, tag="bias")
            nc.sync.dma_start(bias_sb[:], biasT.ap().rearrange(
                "p (m q) -> p m q", m=MT))
            carry_sb = cpool.tile([P, ROUNDS, KT * BC], mybir.dt.uint8,
                                  tag="carry")
            nc.sync.dma_start(carry_sb[:], carry.ap().rearrange("r p c -> p r c"))
            cinit_sb = cpool.tile([P, ROUNDS, KT * BC], BF16, tag="cinit")
            nc.sync.dma_start(cinit_sb[:], cinit.ap().rearrange("r p c -> p r c"))
            gidx_sb = cpool.tile([P, 1], I32, tag="gidx")
            nc.sync.dma_start(gidx_sb[:], gidx[:])
            sblk_sb = cpool.tile([P, 1], I32, tag="sblk")
            nc.sync.dma_start(sblk_sb[:], sidx_blk[:])
            sfeed_sb = cpool.tile([P, 1], I32, tag="sfeed")
            nc.sync.dma_start(sfeed_sb[:], sidx_feed[:])
            ones_sb = cpool.tile([P, P], BF16, tag="ones")
            nc.vector.memset(ones_sb[:], 1.0)

            # h output tiles, double-buffered by round parity.
            # layout [P, t, m, bc]
            curA = cpool.tile([P, T, MT, BC], BF16, tag="curA")
            curB = cpool.tile([P, T, MT, BC], BF16, tag="curB")
            nc.vector.memset(curA[:], 0.0)
            nc.vector.memset(curB[:], 0.0)

            # xw PSUM tiles: [P, bank(4), t'(8), m(8), bc(8)] fp32.
            # bank = t%2 + 2*(t>=16); t' = (t%16)//2. Consecutive steps hit
            # different banks so ACT(t) reads never collide with MM(t+1)
            # writes; 4 banks per tile, double-buffered = all 8 banks.
            psA = pspool.tile([P, 4, 8, MT, BC], F32, tag="psA")  # t' padded to 8
            psB = pspool.tile([P, 4, 8, MT, BC], F32, tag="psB")  # so each b is a real bank

            xblks = [cpool.tile([P, T, KT, BC], BF16, tag=f"xblk{i}", name=f"xblk{i}")
                     for i in range(2)]
            feeds = [cpool.tile([P, BLK_COLS], BF16, tag=f"feed{i}", name=f"feed{i}")
                     for i in range(2)]

            # ---- projection emission helper -------------------------------
            def proj_jobs(r):
                """Thunks emitting proj/bias MMs for round r's xw.

                Writes ps[r%2]; reads xblk tile of round r. Ordered so the
                always-ready bias MMs come first (they fill PE slots while
                the gathers land), then lo-half proj, then hi-half proj.
                The first bias MM per bank has start=True (clears the bank);
                everything else accumulates and depends on that clear.
                """
                ps = psA if r % 2 == 0 else psB
                xb = xblks[r % 2]
                clear_mm = [None, None, None, None]

                def mk_bias(b, m):
                    # rank-1: lhsT = bias tile (only partition 0 nonzero),
                    # rhs = ones => writes b[m*128+o] to every (t', bc) col
                    def emit():
                        mm = nc.tensor.matmul(
                            ps[:, b, 0:T // 4, m, :],
                            bias_sb[:, m, :],
                            ones_sb[:, :T // 4 * BC],
                            start=(clear_mm[b] is None),
                            stop=False,
                            skip_group_check=True,
                        )
                        if clear_mm[b] is None:
                            clear_mm[b] = mm
                        else:
                            add_dep_helper(mm.ins, clear_mm[b].ins, sync=False,
                                           reason="after bank clear")
                        return mm
                    return emit

                def mk(H, par, m, k):
                    def emit():
                        b = 2 * H + par
                        mm = nc.tensor.matmul(
                            ps[:, b, 0:T // 4, m, :],
                            wx_sb[:, k, m, :],
                            xb[:, (T // 2) * H + par:(T // 2) * (H + 1):2, k, :],
                            start=False,
                            stop=False,
                            skip_group_check=True,
                        )
                        add_dep_helper(mm.ins, clear_mm[b].ins, sync=False,
                                       reason="after bank clear")
                        return mm
                    return emit

                jobs = []
                for b in range(4):
                    for m in range(MT):
                        jobs.append(mk_bias(b, m))
                for H in range(2):
                    for par in range(2):
                        for m in range(MT):
                            for k in range(KT):
                                jobs.append(mk(H, par, m, k))
                return jobs

            # ---- prologue: gather + project block for round 0 -------------
            nc.sync.dma_start(xblks[0][:], ag_init[0].rearrange(
                "p (t k c) -> p t k c", t=T, k=KT))
            for j in proj_jobs(0):
                j()

            def contribute(r, half):
                """Scatter this round's half-block + trigger its AG."""
                cur = curA if r % 2 == 0 else curB
                lo, hi = half * HC, (half + 1) * HC
                tlo, thi = half * (T // 2), (half + 1) * (T // 2)
                sc_blk = nc.gpsimd.indirect_dma_start(
                    out=ag_ins[half][r][:],
                    out_offset=bass.IndirectOffsetOnAxis(
                        ap=sblk_sb[:, :1], axis=0),
                    in_=cur[:, tlo:thi, :, :].rearrange("p t m c -> p (t m c)"),
                    in_offset=None,
                )
                cc = nc.gpsimd.collective_compute(
                    "AllGather",
                    mybir.AluOpType.bypass,
                    replica_groups=[list(range(N_CORES))],
                    ins=[ag_ins[half][r][:]],
                    outs=[ag_outs[half][r][:]],
                )
                add_dep_helper(cc.ins, sc_blk.ins, sync=True,
                               reason="AG after block scatter")
                add_dep_helper(cc.ins, sc_feeds[half].ins, sync=True,
                               reason="AG after feed scatter")
                cc_lists[half].append(cc)

            cc_lists = [[], []]
            sc_feeds = [None, None]
            for r in range(ROUNDS):
                cur = curA if r % 2 == 0 else curB
                prev = curB if r % 2 == 0 else curA
                ps = psA if r % 2 == 0 else psB

                # ---- early, off critical path ----
                # feed halves for this round's AG contributions
                if r < NAG:
                    nc.sync.dma_start(feeds[r % 2][:], x0t[r])
                    for half in range(2):
                        sc_feeds[half] = nc.gpsimd.indirect_dma_start(
                            out=ag_ins[half][r][:],
                            out_offset=bass.IndirectOffsetOnAxis(
                                ap=sfeed_sb[:, :1], axis=0),
                            in_=feeds[r % 2][:, half * HC:(half + 1) * HC],
                            in_offset=None,
                        )
                # gather next round's input block (two halves)
                if r + 1 < ROUNDS:
                    nxb = xblks[(r + 1) % 2]
                    if r + 1 < LAG:
                        nc.sync.dma_start(nxb[:], ag_init[r + 1].rearrange(
                            "p (t k c) -> p t k c", t=T, k=KT))
                    else:
                        for half in range(2):
                            tlo, thi = half * (T // 2), (half + 1) * (T // 2)
                            g = nc.gpsimd.indirect_dma_start(
                                out=nxb[:, tlo:thi, :, :].rearrange(
                                    "p t k c -> p (t k c)"),
                                out_offset=None,
                                in_=ag_outs[half][r - 1][:],
                                in_offset=bass.IndirectOffsetOnAxis(
                                    ap=gidx_sb[:, :1], axis=0),
                            )
                            add_dep_helper(g.ins, cc_lists[half][r - 1].ins,
                                           sync=True, reason="gather after AG")

                # ---- h_start = carry ? prev block tail : cinit ----
                hstart = hspool.tile([P, KT * BC], BF16, tag="hs")
                nc.vector.tensor_copy(hstart[:], cinit_sb[:, r])
                nc.vector.copy_predicated(
                    hstart[:], carry_sb[:, r],
                    prev[:, T - 1, :, :].rearrange("p m c -> p (m c)"))

                # ---- 32 recurrence steps; proj(r+1) MMs fill every slot ----
                # The scheduler floats independent MMs, so fillers are pinned
                # into their slot with sync=False ordering edges: fillers
                # follow the slot's first step-MM; the next step's first MM
                # follows the slot's last filler. Fillers then execute in the
                # PE FIFO while ACT(t) completes, hiding its latency.
                # Slot ranges keep each filler's gather strictly older than
                # its slot so it can never head-of-line-block the FIFO.
                jobs = proj_jobs(r + 1) if r + 1 < ROUNDS else []
                ji = 0
                anchor = None
                for t in range(T):
                    bnk = t % 2 + 2 * (t >= T // 2)
                    reg = ps[:, bnk, (t % (T // 2)) // 2, :, :]
                    first_mm = None
                    for k in range(KT):
                        if t == 0:
                            rhs = hstart[:, k * BC:(k + 1) * BC]
                        else:
                            rhs = cur[:, t - 1, k, :]
                        for m in range(MT):
                            mm = nc.tensor.matmul(
                                reg[:, m, :],
                                wh_sb[:, k, m, :],
                                rhs,
                                start=False,
                                stop=False,
                                skip_group_check=True,
                            )
                            if first_mm is None:
                                first_mm = mm
                                if anchor is not None:
                                    add_dep_helper(mm.ins, anchor.ins,
                                                   sync=False,
                                                   reason="after fillers")
                    nc.scalar.activation(cur[:, t, :, :], reg, Tanh)
                    # fillers for this slot: bias in steps 1-4, lo-half proj
                    # in 4-17, hi-half proj from 18 (gather timing safety)
                    nfill = 0
                    if t >= 1:
                        nj = min(16 if t < 3 else 20, len(jobs) - ji)
                        if t < 10:
                            nj = min(nj, 32 + 128 - ji)  # bias+lo only
                        for _ in range(max(nj, 0)):
                            f = jobs[ji]()
                            add_dep_helper(f.ins, first_mm.ins, sync=False,
                                           reason="filler in slot")
                            anchor = f
                            ji += 1
                            nfill += 1
                    # dummy fillers top the slot up so the PE never idles
                    # while ACT(t) completes (also keeps the HAM clock warm).
                    # They accumulate garbage into the previous step's
                    # already-consumed xw region: ACT(t-1) is provably done
                    # (this step's MMs waited on it), ACT(t) reads a
                    # different bank, so no PSUM collision.
                    if t >= 1:
                        bprev = (t - 1) % 2 + 2 * ((t - 1) >= T // 2)
                        dreg = ps[:, bprev, ((t - 1) % (T // 2)) // 2, :, :]
                        for di in range(max(DUM_PER_SLOT - nfill, 0)):
                            f = nc.tensor.matmul(
                                dreg[:, di % MT, :],
                                wh_sb[:, 0, di % MT, :],
                                ones_sb[:, :BC],
                                start=False,
                                stop=False,
                                skip_group_check=True,
                            )
                            add_dep_helper(f.ins, first_mm.ins, sync=False,
                                           reason="dummy filler")
                            anchor = f
                    if t == T // 2 - 1 and r < NAG:
                        contribute(r, 0)   # lo half done -> AG it mid-round
                while ji < len(jobs):
                    f = jobs[ji]()
                    add_dep_helper(f.ins, first_mm.ins, sync=False,
                                   reason="filler in slot")
                    ji += 1

                # ---- write output block (bf16; host converts) ----
                nc.sync.dma_start(
                    out[r], cur[:].rearrange("p t m c -> p (t m c)"))

                if r < NAG:
                    contribute(r, 1)
    nc.compile()
    return nc


def _prep_inputs(X, h0s, W, b):
    """Build the 8 per-core input maps."""
    in_maps = []

    def xb_layout(Xj):
        # [bc, L, d] -> per block [P, (t, k, bc)]
        A = Xj.reshape(BC, NB, T, KT, P)          # [bc, nb, t, k, p]
        A = A.transpose(1, 4, 2, 3, 0)            # [nb, p, t, k, bc]
        return np.ascontiguousarray(A.reshape(NB, P, BLK_COLS)).astype(BF)

    for c in range(N_CORES):
        s, j = c // 2, c % 2
        Wl = np.asarray(W[s], dtype=np.float32)
        Wx, Wh = Wl[:, :D], Wl[:, D:]

        def tiles(M):  # M: [e, d] -> lhsT tiles [p, (k, m, q)]
            A = M.reshape(MT, P, KT, P)           # [m, q, k, p]
            return np.ascontiguousarray(
                A.transpose(3, 2, 0, 1).reshape(P, KT * MT * P)).astype(BF)

        whT = tiles(Wh)
        wxT = tiles(Wx)
        biasT = np.zeros((P, MT, P), np.float32)
        biasT[0] = np.asarray(b[s], np.float32).reshape(MT, P)
        biasT = biasT.reshape(P, MT * P).astype(BF)

        hin = np.asarray(h0s[s, BC * j:BC * (j + 1)], np.float32)  # [bc, d]
        hinit = np.ascontiguousarray(
            hin.reshape(BC, KT, P).transpose(2, 1, 0).reshape(P, KT * BC)
        ).astype(BF)

        carry = np.zeros((ROUNDS, P, KT * BC), np.uint8)
        cinit = np.zeros((ROUNDS, P, KT * BC), BF)
        for r in range(ROUNDS):
            if r > LAG * s:
                carry[r] = 1
            elif r == LAG * s:
                cinit[r] = hinit

        x0t = np.zeros((ROUNDS, P, BLK_COLS), BF)
        ag_init = np.zeros((LAG, P, BLK_COLS), BF)
        if s == 0:
            Xb = xb_layout(np.asarray(X[BC * j:BC * (j + 1)], np.float32))
            ag_init[0] = Xb[0]
            ag_init[1] = Xb[1]
        if s == 3:
            # cores 6,7 carry the stage-0 feed for half j: block r+2 at round r
            Xb = xb_layout(np.asarray(X[BC * j:BC * (j + 1)], np.float32))
            for r in range(ROUNDS):
                if r + 2 < NB:
                    x0t[r] = Xb[r + 2]

        if s == 0:
            gidx = ((6 + c) * RP + np.arange(P, dtype=np.int32)).reshape(P, 1)
        else:
            gidx = ((c - 2) * RP + np.arange(P, dtype=np.int32)).reshape(P, 1)
        if s == 3:
            sidx_blk = np.full((P, 1), P, np.int32)      # junk row
            sidx_feed = np.arange(P, dtype=np.int32).reshape(P, 1)
        else:
            sidx_blk = np.arange(P, dtype=np.int32).reshape(P, 1)
            sidx_feed = np.full((P, 1), P, np.int32)     # junk row

        in_maps.append({
            "whT": whT, "wxT": wxT, "biasT": biasT,
            "carry": carry, "cinit": cinit,
            "gidx": gidx, "sidx_blk": sidx_blk, "sidx_feed": sidx_feed,
            "x0t": x0t, "ag_init": ag_init,
        })
    return in_maps


def _extract(results):
    """Assemble full output [B, L, D] from stage-3 cores (6, 7)."""
    Y = np.empty((B, L, D), np.float32)
    r0 = LAG * (NL - 1)
    for j in range(2):
        o = results[6 + j]["out"][r0:r0 + NB]            # [nb, p, (t m c)]
        o = o.reshape(NB, P, T, MT, BC).astype(np.float32)
        o = o.transpose(4, 0, 2, 3, 1)                   # [bc, nb, t, m, p]
        Y[BC * j:BC * (j + 1)] = o.reshape(BC, L, D)
    return Y


def kernel(X, h0s, W, b, _trace=False):
    from concourse.bass_utils import run_bass_kernel_spmd

    if "nc" not in _cache:
        _cache["nc"] = _build()
    nc = _cache["nc"]
    in_maps = _prep_inputs(np.asarray(X), np.asarray(h0s), np.asarray(W),
                           np.asarray(b))
    res = run_bass_kernel_spmd(nc, in_maps, core_ids=list(range(N_CORES)),
                               trace=_trace)
    _cache["last_results"] = res
    return _extract(res.results)
